# revision 2
# baseline (speedup 1.0000x reference)
"""Trainium2 Bass kernel for nn_BDH_GPU (sparse linear-attention decoder), v2.

Self-contained SPMD program for 8 NeuronCores, shards batch(2) x
head-groups(4), runs via PJRT (axon), gathers output.

v2 layout/schedule vs baseline:
- Layer split into two causal T-chunks (t 0..511 / 512..1023); the chunk-0
  enc-matmul + AllReduce + LN-tail overlap chunk-1 head compute, and the
  chunk-1 AllReduce overlaps the next layer's chunk-0 head compute.
- y@encoder computed in natural [t, d] orientation; LN stats become
  per-partition free-dim reductions (ACT accum_out), the residual+scale is a
  single fused scalar_tensor_tensor, and encoder rows are mean-centered on
  the host so no mean chain is needed on device.
- LN(a) scale rs is folded past the Wy matmul into the gating multiply
  (y = relu(Wy^T a) * (rs*x)), removing the aln tensor entirely.
- vT is produced by PE transposes (tensor.transpose) instead of DMA
  transposes; v lives natural-major (vn/vf), vT only for the Wx/Wy matmuls.
"""
import numpy as np
import ml_dtypes

import concourse.bass as bass
import concourse.tile as tile
import concourse.mybir as mybir
from concourse import bacc, bass2jax
from concourse.masks import make_identity

AF = mybir.ActivationFunctionType
ALU = mybir.AluOpType
FP32 = mybir.dt.float32
BF16 = mybir.dt.bfloat16
ts = bass.ts

D, H, N, VOCAB, L, SD, B, T = 1024, 16, 8192, 32000, 4, 512, 2, 1024
NCORES = 8
NHC = 4           # heads per core
VSH = VOCAB // 4  # vocab shard per core (within batch group) = 8000
VCH = 250         # vocab N-chunk (8 live rtile bufs must fit SBUF)
NVC = VSH // VCH  # 32
EPS = 1e-5
TC = 512          # T-chunk size
NSUB = TC // 128  # 4 t-subchunks per chunk

_CACHE = {}


def build_program(nlayers=L, repeat=1, do_readout=True, collective=True):
    nc = bacc.Bacc("TRN2", target_bir_lowering=False, debug=False,
                   num_devices=NCORES)
    CDT = BF16
    ADT = BF16  # allreduce dtype

    v0n_f = nc.dram_tensor("v0nf", [T, D], FP32, kind="ExternalInput")
    v0n_c = nc.dram_tensor("v0nc", [T, D], CDT, kind="ExternalInput")
    v0t_c = nc.dram_tensor("v0tc", [D, T], CDT, kind="ExternalInput")
    wx_d = nc.dram_tensor("wx", [NHC, D, SD], CDT, kind="ExternalInput")
    wy_d = nc.dram_tensor("wy", [NHC, D, SD], CDT, kind="ExternalInput")
    enc_d = nc.dram_tensor("enc", [NHC * SD, D], CDT, kind="ExternalInput")
    ro_d = nc.dram_tensor("ro", [D, VSH], CDT, kind="ExternalInput")
    cos_d = nc.dram_tensor("cos", [SD // 2, T], CDT, kind="ExternalInput")
    sin_d = nc.dram_tensor("sin", [SD // 2, T], CDT, kind="ExternalInput")
    msk_d = nc.dram_tensor("msk", [2, 128, 256], CDT, kind="ExternalInput")
    out_d = nc.dram_tensor("logits", [T, VSH], FP32, kind="ExternalOutput")

    with tile.TileContext(nc) as tc:
        with (
            tc.tile_pool(name="res", bufs=1) as res,
            tc.tile_pool(name="act", bufs=1) as act,
            tc.tile_pool(name="wst", bufs=12) as wst,
            tc.tile_pool(name="est", bufs=8) as est,
            tc.tile_pool(name="sml", bufs=2) as sml,
            tc.tile_pool(name="stg", bufs=3) as stg,
            tc.tile_pool(name="psp", bufs=2, space="PSUM") as psp,
            tc.tile_pool(name="dram", bufs=2, space="DRAM") as dram,
        ):
            def P5(nm):
                return psp.tile([128, 512], FP32, tag="p5", bufs=2, name=nm)

            def P2(nm):
                return psp.tile([128, 256], FP32, tag="p2", bufs=2, name=nm)

            def PA(nm):
                return psp.tile([128, 256], FP32, tag="ap2", bufs=2, name=nm)

            def PS(nm):
                return psp.tile([128, 256], FP32, tag="stp", bufs=1, name=nm)

            def PT(nm):
                return psp.tile([128, 1024], CDT, tag="pt", bufs=1, name=nm)

            # ---- constants ----
            cosv, sinv, masks = [], [], []
            for i in range(2):
                ct = res.tile([128, T], CDT, name=f"cos{i}")
                nc.sync.dma_start(ct[:], cos_d[ts(i, 128), :])
                cosv.append(ct)
                st = res.tile([128, T], CDT, name=f"sin{i}")
                nc.sync.dma_start(st[:], sin_d[ts(i, 128), :])
                sinv.append(st)
            for i in range(2):
                mt = res.tile([128, 256], CDT, name=f"msk{i}")
                nc.sync.dma_start(mt[:], msk_d[i])
                masks.append(mt)
            ones = res.tile([128, 128], CDT, name="ones")
            nc.vector.memset(ones[:], 1.0)
            epst = res.tile([128, 1], FP32, name="epst")
            nc.vector.memset(epst[:], EPS)
            ident = res.tile([128, 128], CDT, name="ident")
            make_identity(nc, ident[:])

            def load_v(sfx):
                # vtc first: layer 0's x-matmuls consume it immediately;
                # vnc is first needed at sV (~30us in), vf only at the tail
                vf, vnc, vtc = [], [], []
                for k in range(8):
                    c = res.tile([128, T], CDT, tag=f"vt{k}", name=f"vt{k}_{sfx}")
                    nc.sync.dma_start(c[:], v0t_c[ts(k, 128), :])
                    vtc.append(c)
                for g in range(8):
                    b = res.tile([128, D], CDT, tag=f"vn{g}", name=f"vn{g}_{sfx}")
                    nc.sync.dma_start(b[:], v0n_c[ts(g, 128), :])
                    vnc.append(b)
                for g in range(8):
                    a = res.tile([128, D], FP32, tag=f"vf{g}", name=f"vf{g}_{sfx}")
                    nc.scalar.dma_start(a[:], v0n_f[ts(g, 128), :])
                    vf.append(a)
                return vf, vnc, vtc

            for rep in range(repeat):
                vf, vnc, vtc = load_v(f"r{rep}")
                pending_tail = [None]  # (core_fn, transpose_fn)

                for layer in range(nlayers):
                    lt = f"r{rep}l{layer}"
                    ytiles = {}
                    qh_all = {}
                    for h in range(NHC):
                        qh_all[h] = [act.tile([128, T], CDT, tag=f"qr{i}h{h}",
                                              bufs=1, name=f"qr{i}h{h}_{lt}")
                                     for i in range(4)]
                    # encoder tiles for the whole layer; the DMA issues are
                    # deferred to mid-heads (h==1) so the 4MB of transfers
                    # don't fight the layer-start weight loads for bandwidth
                    ech = []

                    def load_ech(half):
                        for kk in range(8 * half, 8 * half + 8):
                            e = est.tile([128, D], CDT, tag=f"et{kk}", bufs=1,
                                         name=f"e_{lt}k{kk}")
                            eng = nc.scalar if kk % 2 == 0 else nc.sync
                            eng.dma_start(e[:], enc_d[ts(kk, 128), :])
                            ech.append(e)

                    def xrope(c, h, lt=lt, qh_all=qh_all, vtc=vtc):
                        """x = relu(Wx^T @ vT[:, chunk]) and rope -> qh."""
                        tg = f"{lt}c{c}h{h}"
                        csl = ts(c, TC)
                        qh = qh_all[h]
                        wxt = []
                        for k in range(8):
                            w = wst.tile([128, SD], CDT, tag="wtile",
                                         name=f"wx_{tg}k{k}")
                            nc.sync.dma_start(w[:], wx_d[h, ts(k, 128), :])
                            wxt.append(w)
                        xp = []
                        for m in range(4):
                            x = act.tile([128, TC], CDT, tag=f"xp{m}", bufs=2,
                                         name=f"xp{m}_{tg}")
                            xp.append(x)
                            ps = P5(f"xps_{tg}m{m}")
                            for k in range(8):
                                nc.tensor.matmul(
                                    ps[:], wxt[k][:, ts(m, 128)],
                                    vtc[k][:, csl],
                                    start=(k == 0), stop=(k == 7))
                            nc.scalar.activation(out=x[:], in_=ps[:],
                                                 func=AF.Relu)
                        for i in range(2):
                            cs_ = cosv[i][:, csl]
                            sn_ = sinv[i][:, csl]
                            t1 = sml.tile([128, TC], CDT, tag="ropet1", bufs=2,
                                          name=f"t1_{tg}i{i}")
                            nc.gpsimd.tensor_mul(t1[:], xp[i][:], cs_)
                            nc.gpsimd.tensor_mul(qh[i][:, csl], xp[2 + i][:], sn_)
                            nc.vector.tensor_sub(qh[i][:, csl], t1[:],
                                                 qh[i][:, csl])
                            t3 = sml.tile([128, TC], CDT, tag="ropet1", bufs=2,
                                          name=f"t3_{tg}i{i}")
                            nc.gpsimd.tensor_mul(t3[:], xp[i][:], sn_)
                            nc.gpsimd.tensor_mul(qh[2 + i][:, csl],
                                                 xp[2 + i][:], cs_)
                            nc.vector.tensor_add(qh[2 + i][:, csl], t3[:],
                                                 qh[2 + i][:, csl])
                        return xp

                    def attn_y(c, h, xp, inject=None, lt=lt, ytiles=ytiles,
                               qh_all=qh_all, vnc=vnc):
                        tg = f"{lt}c{c}h{h}"
                        qh = qh_all[h]
                        # ---- attention + fused LN scale folded into gate ----
                        af = [act.tile([128, TC], CDT, tag=f"af{d8}", bufs=1,
                                       name=f"af{d8}_{tg}") for d8 in range(8)]
                        for j in (2 * c, 2 * c + 1):
                            jj = j - 2 * c
                            tj = ts(j, 256)
                            nsb = 2 * j + 2
                            sc = [sml.tile([128, 256], CDT, tag=f"sc{i}", bufs=1,
                                           name=f"sc{i}_{tg}j{j}")
                                  for i in range(nsb)]
                            for i in range(nsb):
                                ps = P2(f"scp_{tg}j{j}i{i}")
                                for k in range(4):
                                    nc.tensor.matmul(
                                        ps[:], qh[k][:, ts(i, 128)], qh[k][:, tj],
                                        start=(k == 0), stop=(k == 3))
                                if i >= 2 * j:
                                    nc.vector.tensor_mul(sc[i][:], ps[:],
                                                         masks[i - 2 * j][:])
                                elif i % 2 == 0:
                                    nc.scalar.activation(out=sc[i][:], in_=ps[:],
                                                         func=AF.Copy)
                                else:
                                    nc.vector.tensor_copy(sc[i][:], ps[:])
                            if inject is not None:
                                # next head's x-matmuls/rope fill the PE while
                                # this head's PA chains wait on evacuations
                                inject()
                                inject = None
                            # PA chains with the stp (sum of squares) matmuls
                            # staggered two chains behind, so the PE never
                            # waits on the af-copy -> square round trip
                            stp = PS(f"stp_{tg}j{j}")
                            sqs = {}

                            def stp_mm(d8):
                                nc.tensor.matmul(stp[:], ones[:], sqs[d8][:],
                                                 start=(d8 == 0), stop=(d8 == 7))

                            for d8 in range(8):
                                ps = PA(f"ap_{tg}j{j}d{d8}")
                                for i in range(nsb):
                                    nc.tensor.matmul(
                                        ps[:], vnc[i][:, ts(d8, 128)], sc[i][:],
                                        start=(i == 0), stop=(i == nsb - 1))
                                if d8 % 2 == 0:
                                    nc.scalar.activation(
                                        out=af[d8][:, ts(jj, 256)], in_=ps[:],
                                        func=AF.Copy)
                                else:
                                    nc.vector.tensor_copy(
                                        af[d8][:, ts(jj, 256)], ps[:])
                                sq = sml.tile([128, 256], CDT, tag="sq", bufs=3,
                                              name=f"sq_{tg}j{j}d{d8}")
                                sq_eng = nc.vector if d8 % 2 == 0 else nc.gpsimd
                                sq_eng.tensor_mul(sq[:], af[d8][:, ts(jj, 256)],
                                                  af[d8][:, ts(jj, 256)])
                                sqs[d8] = sq
                                if d8 >= 3:
                                    stp_mm(d8 - 3)
                            stp_mm(5)
                            stp_mm(6)
                            stp_mm(7)
                            rs = sml.tile([128, 256], FP32, tag="rs", bufs=1,
                                          name=f"rs_{tg}j{j}")
                            nc.scalar.activation(out=rs[:], in_=stp[:],
                                                 func=AF.Sqrt, bias=epst[:],
                                                 scale=1.0 / D)
                            nc.vector.reciprocal_approx_fast(rs[:], rs[:])
                            # fold rs into the gate: xp *= rs (in place)
                            for m in range(4):
                                nc.vector.tensor_mul(xp[m][:, ts(jj, 256)],
                                                     xp[m][:, ts(jj, 256)], rs[:])

                        # ---- z = Wy^T @ a ; y = relu(z) * (rs*x) ----
                        wyt = []
                        for k in range(8):
                            w = wst.tile([128, SD], CDT, tag="wtile",
                                         name=f"wy_{tg}k{k}")
                            nc.sync.dma_start(w[:], wy_d[h, ts(k, 128), :])
                            wyt.append(w)
                        yt = [act.tile([128, TC], CDT, tag=f"y{h}_{m}", bufs=1,
                                       name=f"y{h}_{m}_{tg}") for m in range(4)]
                        for m in range(4):
                            ps = P5(f"zps_{tg}m{m}")
                            for k in range(8):
                                nc.tensor.matmul(
                                    ps[:], wyt[k][:, ts(m, 128)], af[k][:],
                                    start=(k == 0), stop=(k == 7))
                            rl = sml.tile([128, TC], CDT, tag="rl", bufs=1,
                                          name=f"rl_{tg}m{m}")
                            nc.scalar.activation(out=rl[:], in_=ps[:],
                                                 func=AF.Relu)
                            nc.vector.tensor_mul(yt[m][:], rl[:], xp[m][:])
                        ytiles[(c, h)] = yt

                    def enc_ar(c, lt=lt, ytiles=ytiles, ech=ech):
                        tg = f"{lt}c{c}"
                        ar_in = dram.tile([TC, D], ADT, tag=f"ar_in{c}",
                                          name=f"ari_{tg}")
                        ar_out = dram.tile([TC, D], ADT, tag=f"ar_out{c}",
                                           name=f"aro_{tg}")
                        for tsub in range(NSUB):
                            for dh in range(2):
                                ps = P5(f"ep_{tg}t{tsub}d{dh}")
                                for kk in range(16):
                                    nc.tensor.matmul(
                                        ps[:],
                                        ytiles[(c, kk // 4)][kk % 4][:, ts(tsub, 128)],
                                        ech[kk][:, ts(dh, 512)],
                                        start=(kk == 0), stop=(kk == 15))
                                so = stg.tile([128, 512], ADT, tag="so", bufs=2,
                                              name=f"so_{tg}t{tsub}d{dh}")
                                nc.scalar.activation(out=so[:], in_=ps[:],
                                                     func=AF.Copy)
                                nc.sync.dma_start(
                                    ar_in[ts(tsub, 128), ts(dh, 512)], so[:])
                        if collective:
                            nc.gpsimd.collective_compute(
                                "AllReduce", ALU.add,
                                replica_groups=[[0, 1, 2, 3], [4, 5, 6, 7]],
                                ins=[ar_in.opt()], outs=[ar_out.opt()])
                            return ar_out
                        return ar_in

                    def make_tail(c, ar_out, lt=lt, vf=vf, vnc=vnc, vtc=vtc):
                        tg = f"{lt}c{c}"

                        def core(anchors=None):
                            for tsub in range(NSUB):
                                g = c * NSUB + tsub
                                wb = sml.tile([128, D], ADT, tag="wb", bufs=2,
                                              name=f"wb_{tg}t{tsub}")
                                if anchors is not None:
                                    # tiny copy pins the wb DMA issue after a
                                    # late-heads tile exists, so the scheduler
                                    # can't park it on the sync queue while
                                    # the AllReduce is still in flight
                                    anc = anchors[tsub % len(anchors)]
                                    nc.vector.tensor_copy(wb[0:1, 0:1], anc)
                                nc.sync.dma_start(wb[:],
                                                  ar_out[ts(tsub, 128), :])
                                ssq = sml.tile([128, 1], FP32, tag="st1", bufs=4,
                                               name=f"ssq_{tg}t{tsub}")
                                scr = sml.tile([128, D], CDT, tag="scr", bufs=2,
                                               name=f"scr_{tg}t{tsub}")
                                nc.vector.scalar_tensor_tensor(
                                    out=scr[:], in0=wb[:], scalar=1.0,
                                    in1=wb[:], op0=ALU.mult, op1=ALU.mult,
                                    accum_out=ssq[:])
                                rsw = sml.tile([128, 1], FP32, tag="st2", bufs=4,
                                               name=f"rsw_{tg}t{tsub}")
                                nc.scalar.activation(out=rsw[:], in_=ssq[:],
                                                     func=AF.Sqrt, bias=epst[:],
                                                     scale=1.0 / D)
                                nc.vector.reciprocal(rsw[:], rsw[:])
                                # s = u + v = wb*rsw + vf  (in place into vf)
                                nc.vector.scalar_tensor_tensor(
                                    out=vf[g][:], in0=wb[:], scalar=rsw[:],
                                    in1=vf[g][:], op0=ALU.mult, op1=ALU.add)
                                sss = sml.tile([128, 1], FP32, tag="st1", bufs=4,
                                               name=f"sss_{tg}t{tsub}")
                                scr2 = sml.tile([128, D], CDT, tag="scr", bufs=2,
                                                name=f"scr2_{tg}t{tsub}")
                                nc.scalar.activation(out=scr2[:], in_=vf[g][:],
                                                     func=AF.Square,
                                                     accum_out=sss[:])
                                rss = sml.tile([128, 1], FP32, tag="st2", bufs=4,
                                               name=f"rss_{tg}t{tsub}")
                                nc.scalar.activation(out=rss[:], in_=sss[:],
                                                     func=AF.Sqrt, bias=epst[:],
                                                     scale=1.0 / D)
                                nc.vector.reciprocal(rss[:], rss[:])
                                nc.vector.tensor_scalar_mul(vf[g][:], vf[g][:],
                                                            rss[:])
                                nc.scalar.activation(out=vnc[g][:], in_=vf[g][:],
                                                     func=AF.Copy)

                        def transposes():
                            for tsub in range(NSUB):
                                g = c * NSUB + tsub
                                pt = PT(f"pt_{tg}t{tsub}")
                                for k in range(8):
                                    nc.tensor.transpose(pt[:, ts(k, 128)],
                                                        vnc[g][:, ts(k, 128)],
                                                        ident[:])
                                for k in range(8):
                                    if k % 2 == 0:
                                        nc.vector.tensor_copy(
                                            vtc[k][:, ts(g, 128)],
                                            pt[:, ts(k, 128)])
                                    else:
                                        nc.scalar.activation(
                                            out=vtc[k][:, ts(g, 128)],
                                            in_=pt[:, ts(k, 128)], func=AF.Copy)

                        return core, transposes

                    def heads(c):
                        """Head loop with x/rope software-pipelined one ahead;
                        returns after all 4 head blocks are emitted."""
                        xps = {0: xrope(c, 0)}
                        for h in range(NHC):
                            def inject(h=h):
                                if h + 1 < NHC:
                                    xps[h + 1] = xrope(c, h + 1)
                                if c == 0 and h == 1:
                                    load_ech(0)
                                if c == 0 and h == 2:
                                    load_ech(1)
                                if c == 0 and h == 2 and pending_tail[0] is not None:
                                    pc, ptr = pending_tail[0]
                                    pc(anchors=[qh_all[2][0][0:1, 0:1],
                                                qh_all[3][0][0:1, 0:1]])
                                    ptr()
                                    pending_tail[0] = None
                            attn_y(c, h, xps.pop(h), inject=inject)

                    heads(0)
                    ar0 = enc_ar(0)
                    heads(1)
                    tail0_core, tail0_tr = make_tail(0, ar0)
                    tail0_core(anchors=[qh_all[2][0][0:1, TC:TC + 1],
                                        qh_all[3][0][0:1, TC:TC + 1]])
                    ar1 = enc_ar(1)
                    tail0_tr()
                    t1c, t1t = make_tail(1, ar1)
                    pending_tail[0] = (t1c, t1t)

                # flush last layer's c1 tail before readout / next rep
                if pending_tail[0] is not None:
                    pc, ptr = pending_tail[0]
                    pc()
                    ptr()
                    pending_tail[0] = None

                # ---- readout: logits = v^T @ readout_shard ----
                if do_readout and rep == repeat - 1:
                    for nn_ in range(NVC):
                        rot = []
                        for k in range(8):
                            rtile = wst.tile([128, VCH], CDT, tag="rtile", bufs=8,
                                             name=f"ro_n{nn_}k{k}")
                            nc.sync.dma_start(
                                rtile[:], ro_d[ts(k, 128), ts(nn_, VCH)])
                            rot.append(rtile)
                        for m in range(8):
                            ps = P5(f"rps_n{nn_}m{m}")
                            for k in range(8):
                                nc.tensor.matmul(ps[:, 0:VCH],
                                                 vtc[k][:, ts(m, 128)], rot[k][:],
                                                 start=(k == 0), stop=(k == 7))
                            ot = stg.tile([128, VCH], FP32, tag="ot", bufs=1,
                                          name=f"ot_n{nn_}m{m}")
                            if m % 2 == 0:
                                nc.vector.tensor_copy(ot[:], ps[:, 0:VCH])
                            else:
                                nc.scalar.activation(out=ot[:], in_=ps[:, 0:VCH],
                                                     func=AF.Copy)
                            nc.sync.dma_start(
                                out_d[ts(m, 128), ts(nn_, VCH)], ot[:])
    nc.compile()
    return nc


def host_prep(inputs):
    idx = np.asarray(inputs["idx"])
    wte = np.asarray(inputs["wte"], np.float32)
    enc = np.asarray(inputs["encoder"], np.float32)
    dx = np.asarray(inputs["decoder_x"], np.float32)
    dy = np.asarray(inputs["decoder_y"], np.float32)
    ro = np.asarray(inputs["readout"], np.float32)
    bf = ml_dtypes.bfloat16

    perm = np.concatenate([np.arange(0, SD, 2), np.arange(1, SD, 2)])
    Wx = np.ascontiguousarray(dx[:, :, perm])                       # [H, D, SD]
    Wy = np.ascontiguousarray(dy[:, :, perm])
    # center encoder rows over d: y@enc'' is exactly mean-free per token
    enc2 = enc - enc.mean(axis=-1, keepdims=True)
    encp = np.ascontiguousarray(enc2.reshape(H, SD, D)[:, perm, :])  # [H, SD, D]

    g = wte[idx]                                                    # [B, T, D]
    m = g.mean(-1, keepdims=True)
    var = ((g - m) ** 2).mean(-1, keepdims=True)
    v0 = (g - m) / np.sqrt(var + EPS)

    inv_freq = 1.0 / (10000.0 ** (np.arange(0, SD, 2, dtype=np.float32) / SD))
    freqs = np.arange(T, dtype=np.float32)[None, :] * inv_freq[:, None]
    cosT = np.cos(freqs).astype(np.float32)                         # [SD/2, T]
    sinT = np.sin(freqs).astype(np.float32)

    ss, tt = np.mgrid[0:128, 0:256]
    msk = np.stack([(tt > ss), (tt > ss + 128)]).astype(np.float32)

    in_maps = []
    for c in range(NCORES):
        b, hs = c // 4, c % 4
        hsl = slice(4 * hs, 4 * hs + 4)
        v0b = np.ascontiguousarray(v0[b])
        in_maps.append({
            "v0nf": v0b,
            "v0nc": v0b.astype(bf),
            "v0tc": np.ascontiguousarray(v0b.T).astype(bf),
            "wx": Wx[hsl].astype(bf),
            "wy": Wy[hsl].astype(bf),
            "enc": np.ascontiguousarray(encp[hsl].reshape(NHC * SD, D)).astype(bf),
            "ro": np.ascontiguousarray(ro[:, VSH * hs: VSH * (hs + 1)]).astype(bf),
            "cos": cosT.astype(bf),
            "sin": sinT.astype(bf),
            "msk": msk.astype(bf),
        })
    return in_maps


def make_runner(nc, n_cores=NCORES):
    import jax
    from jax.sharding import Mesh, PartitionSpec
    from jax.experimental.shard_map import shard_map

    bass2jax.install_neuronx_cc_hook()
    partition_name = nc.partition_id_tensor.name if nc.partition_id_tensor else None
    in_names, out_names, out_avals, zero_shapes = [], [], [], []
    for alloc in nc.m.functions[0].allocations:
        if not isinstance(alloc, mybir.MemoryLocationSet):
            continue
        name = alloc.memorylocations[0].name
        if alloc.kind == "ExternalInput":
            if name != partition_name:
                in_names.append(name)
        elif alloc.kind == "ExternalOutput":
            shape = tuple(alloc.tensor_shape)
            dtype = mybir.dt.np(alloc.dtype)
            out_names.append(name)
            out_avals.append(jax.core.ShapedArray(shape, dtype))
            zero_shapes.append((shape, dtype))
    n_params, n_outs = len(in_names), len(out_avals)
    all_in = list(in_names) + list(out_names)
    if partition_name is not None:
        all_in.append(partition_name)

    def _body(*args):
        operands = list(args)
        if partition_name is not None:
            operands.append(bass2jax.partition_id_tensor())
        return tuple(bass2jax._bass_exec_p.bind(
            *operands, out_avals=tuple(out_avals), in_names=tuple(all_in),
            out_names=tuple(out_names), lowering_input_output_aliases=(),
            sim_require_finite=True, sim_require_nnan=True, nc=nc))

    devices = jax.devices()[:n_cores]
    mesh = Mesh(np.asarray(devices), ("core",))
    f = jax.jit(
        shard_map(_body, mesh=mesh,
                  in_specs=(PartitionSpec("core"),) * (n_params + n_outs),
                  out_specs=(PartitionSpec("core"),) * n_outs, check_rep=False),
        keep_unused=True)

    def prep(in_maps):
        concat = [np.concatenate([np.asarray(in_maps[c][k])
                                  for c in range(n_cores)], axis=0)
                  for k in in_names]
        zeros = [np.zeros((n_cores * s[0], *s[1:]), dt) for (s, dt) in zero_shapes]
        return [jax.device_put(x) for x in concat + zeros]

    def run(dev_args):
        outs = f(*dev_args)
        jax.block_until_ready(outs)
        return outs

    def split(outs):
        return [{name: np.asarray(outs[i]).reshape(n_cores, *out_avals[i].shape)[c]
                 for i, name in enumerate(out_names)} for c in range(n_cores)]

    return run, prep, split


def kernel(**inputs) -> np.ndarray:
    if "prog" not in _CACHE:
        nc = build_program()
        _CACHE["prog"] = nc
        _CACHE["runner"] = make_runner(nc)
    run, prep, split = _CACHE["runner"]
    in_maps = host_prep(inputs)
    args = prep(in_maps)
    res = split(run(args))
    out = np.zeros((B, T, VOCAB), np.float32)
    for c in range(NCORES):
        b, hs = c // 4, c % 4
        out[b, :, VSH * hs: VSH * (hs + 1)] = res[c]["logits"]
    return out
